# revision 33
# baseline (speedup 1.0000x reference)
"""Causal self-attention (S=2048, B=2, D=1024, H=16) on 8 TRN2 NeuronCores.

Sharding: megatron-style head parallelism. Each core owns 2 heads (128 of the
1024 model dims): Wq/Wk/Wv column-sharded, Wo row-sharded; every core reads the
full x, computes its heads' attention plus its partial output projection, and
the host sums the 8 partial outputs.

v3 (on top of v2's bf16 dataflow, packed score strips, [token,dim] V, fused
exp, PE-interleaved schedule):
- startup: chunk-0 x arrives as four independently-tracked quarter tiles on
  the sync queue while wq loads on the scalar queue and the other weights on
  gpsimd, so the first projection matmul fires ~3us earlier.
- driver: phases alternate batches ((0,i) then (1,i)); every attention phase
  hosts exactly one later chunk's projections plus output projections as PE
  filler, sized to each phase's exp deficit, so the PE stays dense and the
  HAM clock gate holds 2.4 GHz through the body of the kernel.
- out-projection second casts run on ACT in exp-slack phases; tail bundles
  draw PSUM from the (dead) attention pools for a deeper drain pipe; warm
  keepers read a long-dead tile so they are not serialized behind the final
  normalization.
- softmax: PV's ones-column sits at column 0 (PSUM partition 0) so the
  custom-DVE reciprocal reads the row sums straight from PSUM; each phase's
  normalize-multiplies are deferred into the next phase's filler stream so
  they never jam the in-order DVE ahead of the Q/K casts that recycle PSUM.
- gpsimd issues no DMAs, so its fixed ~4us software-DGE drain overlaps the
  tail; bridge warm-matmuls emitted right after the last PV hold the HAM
  clock through the final normalization.
"""

import numpy as np

import concourse.bass as bass
import concourse.mybir as mybir
import concourse.tile as tile
from concourse import bacc, bass_utils

S, B, D = 2048, 2, 1024
NCORES = 8
HPC = 2                # heads per core
HD = 64
TOK = S * B            # 4096 tokens, batch-major: token = b*S + s
KCH = D // 128         # 8 contraction chunks for the projections
ACH = 512              # phase A token-chunk width
NAC = TOK // ACH       # 8 token chunks (4 per batch)
SCH = 512              # s-chunk width (phase B)
NSC = S // SCH         # 4 s-chunks per batch
TBLK = 128             # t-block width
NTB = S // TBLK        # 16 t-blocks per batch

F32 = mybir.dt.float32
BF16 = mybir.dt.bfloat16


def build_program():
    nc = bacc.Bacc("TRN2", target_bir_lowering=False, debug=False, num_devices=NCORES,
                   num_swdge_queues=4)

    xT = nc.dram_tensor("xT", (128, NAC, KCH, ACH), BF16, kind="ExternalInput")
    wqT = nc.dram_tensor("wqT", (128, KCH, 128), BF16, kind="ExternalInput")
    wkT = nc.dram_tensor("wkT", (128, KCH, 128), BF16, kind="ExternalInput")
    wvT = nc.dram_tensor("wvT", (128, KCH, 128), BF16, kind="ExternalInput")
    woS = nc.dram_tensor("woS", (128, KCH, 128), BF16, kind="ExternalInput")
    msk = nc.dram_tensor("msk", (128, TBLK), BF16, kind="ExternalInput")
    idn = nc.dram_tensor("idn", (128, 128), BF16, kind="ExternalInput")
    out = nc.dram_tensor("out", (TOK, D), BF16, kind="ExternalOutput")

    with tile.TileContext(nc) as tc:
        with (
            tc.tile_pool(name="persist", bufs=1) as persist,
            tc.tile_pool(name="xt", bufs=5) as xtp,
            tc.tile_pool(name="pt", bufs=6) as ptp,
            tc.tile_pool(name="small", bufs=4) as smallp,
            tc.tile_pool(name="outsb", bufs=4) as outp,
            tc.tile_pool(name="fill_ps", bufs=2, space="PSUM") as fill_ps,
            tc.tile_pool(name="st_ps", bufs=2, space="PSUM") as st_ps,
            tc.tile_pool(name="ap_ps", bufs=2, space="PSUM") as ap_ps,
        ):
            # ---- persistent tiles
            wq_sb = persist.tile([128, KCH, 128], BF16)
            wk_sb = persist.tile([128, KCH, 128], BF16)
            wv_sb = persist.tile([128, KCH, 128], BF16)
            wo_full = persist.tile([128, KCH, 128], BF16)
            wo_sb = wo_full.rearrange("p o m -> p (o m)")
            msk_sb = persist.tile([128, TBLK], BF16)
            idn_sb = persist.tile([128, 128], BF16)
            qt_c = [persist.tile([128, ACH], BF16, name=f"qt_c{g}") for g in range(NAC)]
            kt_c = [persist.tile([128, ACH], BF16, name=f"kt_c{g}") for g in range(NAC)]
            # V: [t-part, b, t-block, head, 128]; col 0 = 1.0 (PV sum row at
            # PSUM partition 0 where the custom-DVE reciprocal can read it),
            # V data at cols 64:128 (a 64-partition DVE access must start at
            # partition 0 or 64)
            v_sb = persist.tile([128, B, NTB, HPC, 128], BF16)
            atn_sb = persist.tile([128, TOK], BF16)
            # chunk-0 x as 4 separate quarter tiles: each projection matmul
            # waits only on its own 256KB quarter, not the full 1MB chunk
            xt0q = [persist.tile([128, 2, ACH], BF16, name=f"xt0q{qi}")
                    for qi in range(4)]

            # ---- startup DMAs: x chunk-0 quarters alternate sync/scalar
            # queues (with wq interleaved) so projection o-steps stream in
            # arrival order; other weights on gpsimd
            # Three parallel startup channels: wq on scalar, x chunk-0
            # quarters on sync, the other weights on gpsimd. gpsimd's only
            # DMAs are these startup ones (done by ~10us), so its slow
            # fixed-cost software-DGE drain still runs concurrently with the
            # tail instead of gating the final barrier.
            nc.scalar.dma_start(wq_sb[:, 0:KCH // 2, :], wqT[:, 0:KCH // 2, :])
            nc.scalar.dma_start(wq_sb[:, KCH // 2:KCH, :], wqT[:, KCH // 2:KCH, :])
            for qi in range(4):
                nc.sync.dma_start(xt0q[qi], xT[:, 0, 2 * qi:2 * qi + 2, :])
            nc.gpsimd.dma_start(wk_sb[:, 0:KCH // 2, :], wkT[:, 0:KCH // 2, :])
            nc.gpsimd.dma_start(wv_sb[:, 0:KCH // 2, :], wvT[:, 0:KCH // 2, :])
            nc.gpsimd.dma_start(wk_sb[:, KCH // 2:KCH, :], wkT[:, KCH // 2:KCH, :])
            nc.gpsimd.dma_start(wv_sb[:, KCH // 2:KCH, :], wvT[:, KCH // 2:KCH, :])
            nc.gpsimd.dma_start(idn_sb, idn[:, :])
            nc.gpsimd.dma_start(msk_sb, msk[:, :])
            nc.gpsimd.dma_start(wo_full, woS[:, :, :])
            nc.vector.memset(v_sb[:, :, :, :, 0:HD], 0.0)
            nc.vector.memset(v_sb[:, :, :, :, 0:1], 1.0)

            # ---- phase A: one chunk (512 tokens) -> list of PE-work bundles
            def phase_a_bundles(g, with_dma=True):
                b, tc4 = divmod(g, NAC // B)
                cell = {}

                def xsl(o):
                    if g == 0:
                        return xt0q[o // 2][:, o % 2, :]
                    return cell["xt"][:, o, :]

                def xsl_t(o, tsl):
                    if g == 0:
                        return xt0q[o // 2][:, o % 2, tsl]
                    return cell["xt"][:, o, tsl]

                def bndl_dma():
                    xt = xtp.tile([128, KCH, ACH], BF16, tag="xt", name="xt")
                    cell["xt"] = xt
                    half = KCH // 2
                    nc.sync.dma_start(xt[:, 0:half, :], xT[:, g, 0:half, :])
                    nc.sync.dma_start(xt[:, half:KCH, :], xT[:, g, half:KCH, :])

                def bndl_q():
                    ps = fill_ps.tile([128, ACH], F32, tag="fill", name="ps_q")
                    for o in range(KCH):
                        nc.tensor.matmul(ps, wq_sb[:, o, :], xsl(o),
                                         start=(o == 0), stop=(o == KCH - 1))
                    nc.vector.tensor_copy(qt_c[g][:, :], ps)

                def bndl_k():
                    ps = fill_ps.tile([128, ACH], F32, tag="fill", name="ps_k")
                    for o in range(KCH):
                        nc.tensor.matmul(ps, wk_sb[:, o, :], xsl(o),
                                         start=(o == 0), stop=(o == KCH - 1))
                    nc.vector.tensor_copy(kt_c[g][:, :], ps)

                def bndl_v(u):
                    vp = fill_ps.tile([128, TBLK], F32, tag="fill", name="ps_v")
                    tsl = slice(u * TBLK, (u + 1) * TBLK)
                    for o in range(KCH):
                        nc.tensor.matmul(vp, xsl_t(o, tsl), wv_sb[:, o, :],
                                         start=(o == 0), stop=(o == KCH - 1))
                    jb = tc4 * (ACH // TBLK) + u
                    nc.vector.tensor_copy(
                        v_sb[:, b, jb, :, HD:2 * HD],
                        vp.rearrange("p (h e) -> p h e", h=HPC),
                    )

                bundles = [bndl_dma] if with_dma else []
                return bundles + [bndl_q, bndl_k] + \
                    [lambda u=u: bndl_v(u) for u in range(ACH // TBLK)]

            # ---- phase C: one t-block output projection -> one bundle
            # scalar2=True puts the second PSUM->SBUF cast on the ACT engine
            # (for bundles placed in exp-slack phases) so filler chains are
            # not gated on a saturated DVE
            def phase_c_bundle(b, tk, scalar2=False, tail=False, last=False):
                def bndl():
                    base = b * S
                    t_sl = slice(base + tk * TBLK, base + (tk + 1) * TBLK)
                    ob = outp.tile([128, D], BF16, tag="ob")
                    for n in range(D // 512):
                        c_sl = slice(n * 512, (n + 1) * 512)
                        # tail bundles draw PSUM from the attention pools
                        # (dead after the last PV) for a deeper drain pipe
                        pool = (fill_ps, ap_ps)[n] if tail else fill_ps
                        tag = ("fill", "ap")[n] if tail else "fill"
                        op = pool.tile([128, 512], F32, tag=tag, name="op")
                        nc.tensor.matmul(op, atn_sb[:, t_sl], wo_sb[:, c_sl],
                                         start=True, stop=True)
                        if n == 1 and (scalar2 or tail):
                            nc.scalar.copy(ob[:, c_sl], op)
                        else:
                            nc.vector.tensor_copy(ob[:, c_sl], op)
                        if tail:
                            # per-half DMA so the final transfer starts as
                            # soon as its own cast lands; all issues on sync
                            # so scalar's COPY chain is never blocked
                            nc.sync.dma_start(out[t_sl, c_sl], ob[:, c_sl])
                    if not tail:
                        nc.sync.dma_start(out[t_sl, :], ob)
                return bndl

            # ---- phase B: one s-chunk of attention, interleaving fillers
            def phase_b_chunk(b, i, fillers, fast_norm=False, defer_norm=True,
                              pe_bridge=()):
                jmax = (i + 1) * (SCH // TBLK)
                s_sl = slice(b * S + i * SCH, b * S + (i + 1) * SCH)
                nfill = len(fillers)
                fi = 0
                pts = {}

                def scores(j):
                    di = j - i * 4
                    off = max(di, 0) * TBLK
                    ch = b * (NAC // B) + j // 4
                    tsl = slice((j % 4) * TBLK, (j % 4 + 1) * TBLK)
                    stp = st_ps.tile([128, HPC, SCH], F32, tag="st")
                    for h in range(HPC):
                        hsl = slice(h * HD, (h + 1) * HD)
                        nc.tensor.matmul(stp[:, h, off:SCH], kt_c[ch][hsl, tsl],
                                         qt_c[b * (NAC // B) + i][hsl, off:SCH],
                                         start=True, stop=(di < 0))
                    if di >= 0:
                        # accumulate -1e5 strict-upper-tri into the diagonal
                        # band on the PE: exp then yields exact zeros there
                        for h in range(HPC):
                            nc.tensor.matmul(stp[:, h, off:off + TBLK],
                                             idn_sb[:, :], msk_sb[:, :],
                                             start=False, stop=True,
                                             skip_group_check=True)
                    pt = ptp.tile([128, HPC, SCH], BF16, tag="pt")
                    nc.scalar.activation(pt[:, :, off:SCH], stp[:, :, off:SCH],
                                         mybir.ActivationFunctionType.Exp,
                                         scale=0.125)
                    pts[j] = (pt, off)

                def pv(j, aps):
                    pt, off = pts[j]
                    for h in range(HPC):
                        nc.tensor.matmul(aps[h][:, off:SCH], v_sb[:, b, j, h, :],
                                         pt[:, h, off:SCH],
                                         start=(j == 0), stop=(j == jmax - 1))
                    del pts[j]

                aps = [ap_ps.tile([128, SCH], F32, tag="ap", name=f"ap{h}")
                       for h in range(HPC)]
                # front-load two fillers: the first burst has no PV work yet
                while fi < min(2, nfill):
                    fillers[fi]()
                    fi += 1
                for jj in range(0, jmax, 2):
                    # burst of two score pairs: each pair's LDWEIGHTS hides
                    # under the other pair's opposite-row-strip matmul
                    scores(jj)
                    scores(jj + 1)
                    if jj >= 2:
                        pv(jj - 2, aps)
                        pv(jj - 1, aps)
                    # dispense an even share of filler bundles at this step
                    tgt = min(nfill, 2 + (jj + 2) * nfill // jmax)
                    while fi < tgt:
                        fillers[fi]()
                        fi += 1
                pv(jmax - 2, aps)
                pv(jmax - 1, aps)
                for bridge in pe_bridge:
                    bridge()
                rbs = []
                for h in range(HPC):
                    rc = smallp.tile([1, SCH], F32, tag="rc")
                    nc.vector.reciprocal_approx_fast(rc, aps[h][0:1, :])
                    rb = smallp.tile([HD, SCH], F32, tag="rb")
                    nc.gpsimd.partition_broadcast(rb, rc)
                    rbs.append(rb)

                def do_mults():
                    if fast_norm:
                        # tail-latency-critical: normalize per t-block (both
                        # heads) so each tail output projection fires as soon
                        # as its own 128 columns of atn are ready
                        for u in range(SCH // TBLK):
                            usl = slice(u * TBLK, (u + 1) * TBLK)
                            gsl = slice(b * S + i * SCH + u * TBLK,
                                        b * S + i * SCH + (u + 1) * TBLK)
                            for h in range(HPC):
                                nc.vector.tensor_mul(
                                    atn_sb[h * HD:(h + 1) * HD, gsl],
                                    aps[h][HD:2 * HD, usl], rbs[h][:, usl])
                    else:
                        for h in range(HPC):
                            nc.vector.tensor_mul(
                                atn_sb[h * HD:(h + 1) * HD, s_sl],
                                aps[h][HD:2 * HD, :], rbs[h])

                if defer_norm:
                    # the mults run early in the NEXT phase: emitted after
                    # its Q/K casts on the in-order DVE, they no longer jam
                    # the fill_ps slot recycling at phase boundaries
                    return do_mults
                do_mults()
                return None

            # ---------------- driver ----------------
            # prologue: chunk-0 projections; chunk-4 and chunk-1 x DMAs fly
            a = [phase_a_bundles(0, with_dma=False)] + \
                [phase_a_bundles(g) for g in range(1, NAC)]
            for bd in a[0]:
                bd()
            a[4][0]()
            a[1][0]()
            cb0 = [phase_c_bundle(0, tk, scalar2=(tk < 9)) for tk in range(NTB)]
            cb1 = [phase_c_bundle(1, tk, scalar2=(tk < 7), tail=(tk >= 12),
                                  last=(tk == 15))
                   for tk in range(NTB)]

            # ALTERNATING batches: (0,i) then (1,i). Every phase hosts
            # exactly one later chunk's projections as PE filler (chunk c is
            # projected in the last phase before its first consumer), so even
            # the exp-heaviest phases stay PE-dense and the HAM clock gate
            # never drops. Output projections are spread by exp-deficit, with
            # their second PSUM->SBUF cast on ACT only in exp-slack phases.
            def ins(fl, nb):
                return fl[:2] + [nb] + fl[2:] if nb else fl

            def warm():
                # st_ps is free after the last exp, unlike fill_ps whose
                # slots drain behind the final casts
                op = st_ps.tile([128, 512], F32, tag="st", name="warm")
                nc.tensor.matmul(op, qt_c[0][:, 0:128], wo_sb[:, 0:512],
                                 start=True, stop=True)

            n0 = phase_b_chunk(0, 0, a[4][1:] + [a[5][0]])
            n1 = phase_b_chunk(1, 0, ins(a[1][1:] + [a[2][0]], n0))
            n2 = phase_b_chunk(0, 1, ins(a[5][1:] + [a[6][0]] + cb0[0:3], n1))
            n3 = phase_b_chunk(1, 1, ins(a[2][1:] + [a[3][0]] + cb1[0:3], n2))
            n4 = phase_b_chunk(0, 2, ins(a[6][1:] + [a[7][0]] + cb0[3:6]
                                         + cb1[3:5], n3))
            n5 = phase_b_chunk(1, 2, ins(a[3][1:] + cb0[6:9] + cb1[5:7], n4))
            n6 = phase_b_chunk(0, 3, ins(a[7][1:] + cb0[9:12] + cb1[7:9], n5))
            phase_b_chunk(1, 3, ins(cb1[9:11] + cb1[11:12] + cb0[12:16], n6),
                          fast_norm=True, defer_norm=False,
                          pe_bridge=[warm] * 6)
            # warm-keepers: dead matmuls interleaved with the tail output
            # projections bridge the final norm chain's PE idle so the tail
            # runs at the warm clock
            def warm():
                # st_ps is free after the last exp, unlike fill_ps whose
                # slots drain behind the final casts
                op = st_ps.tile([128, 512], F32, tag="st", name="warm")
                nc.tensor.matmul(op, qt_c[0][:, 0:128], wo_sb[:, 0:512],
                                 start=True, stop=True)

            warm()
            warm()
            warm()
            warm()
            for bd in cb1[12:16]:
                warm()
                bd()

    nc.compile()
    return nc


_CACHE = {}


def _get_program():
    if "nc" not in _CACHE:
        _CACHE["nc"] = build_program()
    return _CACHE["nc"]


def _prep_in_maps(x, Wq, Wk, Wv, Wo):
    import ml_dtypes
    bf16 = ml_dtypes.bfloat16

    x = np.asarray(x, dtype=np.float32)
    Wq = np.asarray(Wq, dtype=np.float32)
    Wk = np.asarray(Wk, dtype=np.float32)
    Wv = np.asarray(Wv, dtype=np.float32)
    Wo = np.asarray(Wo, dtype=np.float32)

    # x: (S, B, D) -> xH[p, g, o, tl] = x[s, b, o*128+p], token g*ACH+tl = b*S+s
    xH = np.ascontiguousarray(
        x.transpose(2, 1, 0).reshape(KCH, 128, NAC, ACH).transpose(1, 2, 0, 3)
    ).astype(bf16)

    # additive causal bias for the 128-wide diagonal band: -1e5 where t > s
    p_idx = np.arange(128)[:, None]
    f_idx = np.arange(TBLK)[None, :]
    mskA = np.where(p_idx > f_idx, -1.0e5, 0.0).astype(bf16)
    idnA = np.eye(128, dtype=np.float32).astype(bf16)

    in_maps = []
    for c in range(NCORES):
        sl = slice(c * 128, (c + 1) * 128)
        in_maps.append({
            "xT": xH,
            "wqT": np.ascontiguousarray(
                Wq[sl, :].T.reshape(KCH, 128, 128).transpose(1, 0, 2)).astype(bf16),
            "wkT": np.ascontiguousarray(
                Wk[sl, :].T.reshape(KCH, 128, 128).transpose(1, 0, 2)).astype(bf16),
            "wvT": np.ascontiguousarray(
                Wv[sl, :].T.reshape(KCH, 128, 128).transpose(1, 0, 2)).astype(bf16),
            "woS": np.ascontiguousarray(Wo[:, sl].T.reshape(128, KCH, 128)).astype(bf16),
            "msk": mskA,
            "idn": idnA,
        })
    return in_maps


def run(x, Wq, Wk, Wv, Wo, trace=False):
    nc = _get_program()
    in_maps = _prep_in_maps(x, Wq, Wk, Wv, Wo)
    res = bass_utils.run_bass_kernel_spmd(
        nc, in_maps, core_ids=list(range(NCORES)), trace=trace,
    )
    partial = np.zeros((TOK, D), dtype=np.float32)
    for c in range(NCORES):
        partial += res.results[c]["out"].astype(np.float32)
    full = partial.reshape(B, S, D).transpose(1, 0, 2)  # (S, B, D)
    return np.ascontiguousarray(full), res


def kernel(x, Wq, Wk, Wv, Wo):
    out, _ = run(x, Wq, Wk, Wv, Wo, trace=False)
    return out
